# revision 56
# baseline (speedup 1.0000x reference)
"""3-layer GraphSAGE on 8 trn2 cores — ReduceScatter formulation.

Each core keeps its own node shard's transformed features t = h @ Wl in a
LOCAL DRAM table (fp8 for the two 512-wide layers, fp16x128pad for the
final 64-wide layer), gathers the messages for its OUTGOING edges from
that local table (no cross-core dependency), builds partial
destination-block aggregates for ALL 408 global dst blocks via one-hot
matmuls, writes a [52224, d] fp16 partial table, and a ReduceScatter
(cheap: collective cost scales with its small OUTPUT) sums the 8 partials
and hands each core its own shard's aggregate.  The dst-block space is
split in parts so the RS of part p overlaps the gather/aggregate work of
part p+1; the last part is small so the final (exposed) RS is cheap, and
each part's combine (r-path + inject + relu + transpose) is gated behind
the NEXT part's partial writes so it fills engine idle time inside later
parts' gather windows without stalling the in-order engine queues.

Aggregation runs in fp8 DoubleRow (layers 0/1): 256-edge chunks, one-hot
S built on DVE/GpSimd in f8e4 as [K=128, Ko=2, dst=128] weights against
f8 messages — one PE pass per chunk at 0.5 cycles/row.  Layer 2 keeps
fp16 messages (fp8 there breaks the 2e-2 bar) and consumes the same
256-slot chunk layout as two 128-row fp16 matmuls.  deginv is folded
into the psum->stage copies as a per-partition scale (partitions are dst
nodes), so the post-RS combine needs no separate deginv pass.

Host-side balancer: within each destination shard, nodes are packed into
the 51 dst blocks so that the per-(src-core, block) edge counts stay at
or under 256 (= one DoubleRow chunk).  Without this, the max-over-cores
static chunk count pays a ~1.5x padding tax.
"""

import math
import numpy as np

N_NODES = 50000
N_EDGES = 800000
D_IN = 512
D_HID = 512
D_OUT = 64
N_CORES = 8
NSH = N_NODES // N_CORES          # 6250
NBLK = 51                         # dst blocks per shard (padded: mean 245
                                  # edges per (src-core, block) < 256 cap)
NFREE = NBLK * 128                # 6528
PART_BLOCKS = [13, 13, 13, 10, 2]  # dst-block split per shard
CHUNK = 256                       # edge slots per chunk (fp8 DoubleRow: the
                                  # PE contracts 256 rows per pass)
GATHER_CAP = 8                    # max chunks per dma_gather
WG = 8                            # blocks per batched partial write
RGW = 4                           # blocks per batched agg read (combine)
LOW_LIM_FULL = 32768              # kept for test.py compat (unused)


class Plan:
    pass


def _wrap_idx(v):
    a = np.asarray(v, np.int16).reshape(-1, 16).T   # [16, m/16]
    return np.tile(a, (8, 1))                       # [128, m/16]


def _balance_shard(q, caps):
    """Pack nodes (rows of q: per-src-core in-edge counts) into len(caps)
    blocks so each block's per-core count stays <= 256.  Greedy, heavy
    nodes first.  Returns the new within-shard node order."""
    nb = len(caps)
    nn = q.shape[0]
    loads = np.zeros((nb, q.shape[1]), np.int64)
    cnts = np.zeros(nb, np.int64)
    capv = np.asarray(caps, np.int64)
    order = np.argsort(-q.sum(1), kind="stable")
    assign = np.empty(nn, np.int64)
    for i in order:
        cand = loads + q[i]
        over = np.maximum(cand - 256, 0).sum(1)
        score = over.astype(np.float64) * 1e6 + cand.max(1) \
            + (cnts >= capv) * 1e12
        j = int(np.argmin(score))
        loads[j] += q[i]
        cnts[j] += 1
        assign[i] = j
    assert (cnts <= capv).all() and cnts.sum() == nn
    # local row of each node: bin*128 + rank within bin (blocks may be
    # under-full; pad rows are interspersed)
    o = np.argsort(assign, kind="stable")
    rows = np.empty(nn, np.int64)
    pos = 0
    for b in range(nb):
        c = int(cnts[b])
        rows[o[pos:pos + c]] = b * 128 + np.arange(c)
        pos += c
    return rows


def preprocess(x, edge_index, n_nodes=N_NODES, n_cores=N_CORES, d_in=D_IN,
               low_lim=LOW_LIM_FULL):
    src = np.asarray(edge_index[0], np.int64)
    dst = np.asarray(edge_index[1], np.int64)

    deg = np.bincount(dst, minlength=n_nodes).astype(np.float32)
    deginv_full = (1.0 / np.maximum(deg, 1.0)).astype(np.float32)

    caps = [128] * NBLK

    # ---- balance: assign each node a padded row (shard*NFREE + blk*128+rank)
    csrc_nat = src // NSH
    q = np.zeros((n_nodes, n_cores), np.int64)
    np.add.at(q, (dst, csrc_nat), 1)
    row = np.empty(n_nodes, np.int64)
    for k in range(n_cores):
        rl = _balance_shard(q[k * NSH:(k + 1) * NSH], caps)
        row[k * NSH:(k + 1) * NSH] = k * NFREE + rl

    x = np.asarray(x, np.float32)

    nparts = len(PART_BLOCKS)
    pfirst = np.concatenate([[0], np.cumsum(PART_BLOCKS)]).astype(np.int64)
    prows = [pb * 128 for pb in PART_BLOCKS]

    src_row = row[src]
    dst_row = row[dst]
    kdst = dst_row // NFREE
    r = dst_row - kdst * NFREE
    blocal = r // 128
    within = r - blocal * 128
    part = np.searchsorted(pfirst, blocal, side="right") - 1

    pb_off = np.concatenate([[0], np.cumsum(np.array(PART_BLOCKS) * n_cores)])
    pbs = np.array(PART_BLOCKS)
    gbi = pb_off[part] + kdst * pbs[part] + (blocal - pfirst[part])
    ngb = int(pb_off[-1])

    csrc = src_row // NFREE
    slocal = (src_row - csrc * NFREE).astype(np.int64)

    cnt = np.zeros((n_cores, ngb), np.int64)
    np.add.at(cnt, (csrc, gbi), 1)
    nchunks = np.maximum(-(-cnt.max(axis=0) // CHUNK), 1)
    chunk_base = np.concatenate([[0], np.cumsum(nchunks)]).astype(np.int64)
    totc = int(chunk_base[-1])
    nmax = int(nchunks.max())

    blocks = []   # (part, shard, blk_in_part, chunk0, nchunk)
    i = 0
    for p in range(nparts):
        for k in range(n_cores):
            for b in range(PART_BLOCKS[p]):
                blocks.append((p, k, b, int(chunk_base[i]), int(nchunks[i])))
                i += 1
    assert i == ngb

    groups = []   # (part, blk_lo, blk_hi, chunk0, nchunk)
    gi = 0
    while gi < ngb:
        p = blocks[gi][0]
        lo = gi
        nch = 0
        while gi < ngb and blocks[gi][0] == p and nch + blocks[gi][4] <= GATHER_CAP:
            nch += blocks[gi][4]
            gi += 1
        if gi == lo:
            nch = blocks[gi][4]
            gi += 1
        groups.append((p, lo, gi, blocks[lo][3], nch))

    wgroups = []  # (part, shard, blk_lo, blk_hi)
    gi = 0
    while gi < ngb:
        p, k = blocks[gi][0], blocks[gi][1]
        lo = gi
        while gi < ngb and gi - lo < WG and blocks[gi][0] == p and blocks[gi][1] == k:
            gi += 1
        wgroups.append((p, k, lo, gi))

    plan = Plan()
    plan.nparts = nparts
    plan.part_blocks = list(PART_BLOCKS)
    plan.pfirst = pfirst.tolist()
    plan.prows = prows
    plan.blocks = blocks
    plan.groups = groups
    plan.wgroups = wgroups
    plan.totc = totc
    plan.ngb = ngb
    plan.nmax = nmax
    plan.fingerprint = (tuple(PART_BLOCKS), tuple(nchunks.tolist()),
                        CHUNK, GATHER_CAP, WG)

    # ---- per-core slot tables
    kc = d_in // 128
    per_core = []
    for c in range(n_cores):
        sel = np.nonzero(csrc == c)[0]
        e_g = gbi[sel]
        e_s = slocal[sel]
        e_w = within[sel]
        o = np.argsort(e_g, kind="stable")
        e_g, e_s, e_w = e_g[o], e_s[o], e_w[o]
        grp_start = np.searchsorted(e_g, np.arange(ngb), side="left")
        grp_start_full = np.concatenate([grp_start, [len(e_g)]])
        ranks = np.arange(len(e_g)) - grp_start_full[e_g]
        slots = chunk_base[e_g] * CHUNK + ranks

        idx_slot = np.zeros((totc * CHUNK,), np.int64)
        dst_slot = np.full((totc * CHUNK,), -1.0, np.float32)
        idx_slot[slots] = e_s
        dst_slot[slots] = e_w.astype(np.float32)

        wi = _wrap_idx(idx_slot)                                  # [128, totc*16]
        # dstloc[p, c*2+h] = dst of slot c*CHUNK + h*128 + p   (-1 = empty)
        dl = (dst_slot.reshape(totc, 2, 128).transpose(2, 0, 1)
              .reshape(128, totc * 2).astype(np.float16))

        lrow = row[c * NSH:(c + 1) * NSH] - c * NFREE
        # global deginv table [128, 8*NBLK]: column k*NBLK+b = deginv of the
        # padded node (shard k, block b, partition p); same on every core
        full = np.zeros((n_cores * NFREE,), np.float32)
        full[row] = deginv_full
        dg = full.reshape(n_cores * NBLK, 128).T.astype(np.float32).copy()

        Xp = np.zeros((NFREE, d_in), np.float32)
        Xp[lrow] = x[c * NSH:(c + 1) * NSH]
        xT = np.ascontiguousarray(
            Xp.T.reshape(kc, 128, NFREE).astype(np.float16))

        per_core.append({
            "xT": xT,
            "eidx": np.ascontiguousarray(wi),
            "dstloc": np.ascontiguousarray(dl),
            "deginv": dg,
        })
    return plan, per_core, row


# ---------------------------------------------------------------------------

def build_program(plan, d_in=D_IN, d_hid=D_HID, d_out=D_OUT):
    import os
    dbg = set(os.environ.get("KDBG", "").split(",")) - {""}
    import contextlib
    import concourse.bacc as bacc
    import concourse.tile as tile
    from concourse import mybir
    from concourse.masks import make_identity

    f16 = mybir.dt.float16
    f32 = mybir.dt.float32
    f8 = mybir.dt.float8e4
    i16 = mybir.dt.int16

    nparts = plan.nparts
    totc = plan.totc
    nmax = plan.nmax
    kcs = [d_in // 128, d_hid // 128, d_hid // 128]
    douts = [d_hid, d_hid, d_out]

    nc = bacc.Bacc("TRN2", target_bir_lowering=False, debug=False,
                   num_devices=N_CORES)

    xT_d = nc.dram_tensor("xT", [kcs[0], 128, NFREE], f16, kind="ExternalInput").ap()
    w_d = {}
    for l in range(3):
        kd = [d_in, d_hid, d_hid][l]
        for s in ("l", "r"):
            w_d[(l, s)] = nc.dram_tensor(f"w{s}{l}", [kd, douts[l]], f16,
                                         kind="ExternalInput").ap()
    b_d = [nc.dram_tensor(f"b{l}", [1, douts[l]], f16, kind="ExternalInput").ap()
           for l in range(3)]
    eidx_d = nc.dram_tensor("eidx", [128, totc * 16], i16, kind="ExternalInput").ap()
    dstloc_d = nc.dram_tensor("dstloc", [128, totc * 2], f16, kind="ExternalInput").ap()
    deginv_d = nc.dram_tensor("deginv", [128, N_CORES * NBLK], f32,
                              kind="ExternalInput").ap()
    out_d = nc.dram_tensor("out", [NFREE, d_out], f32, kind="ExternalOutput").ap()

    t8_d = [nc.dram_tensor(f"t8_{l}", [NFREE, d_hid], f8, kind="Internal").ap()
            for l in range(2)]
    t2_d = nc.dram_tensor("t2", [NFREE, 128], f16, kind="Internal").ap()

    part_d = {}
    agg_d = {}
    for l in range(3):
        dd = douts[l]
        for p in range(nparts):
            rows = plan.prows[p]
            part_d[(l, p)] = nc.dram_tensor(
                f"part{l}_{p}", [N_CORES * rows, dd], f16, kind="Internal").ap()
            agg_d[(l, p)] = nc.dram_tensor(
                f"agg{l}_{p}", [rows, dd], f16, kind="Internal").ap()

    rg = [list(range(N_CORES))]

    with tile.TileContext(nc) as tc:
        with contextlib.ExitStack() as ctx:
            cpool = ctx.enter_context(tc.tile_pool(name="const", bufs=1))
            pt_pool = ctx.enter_context(tc.tile_pool(name="pt", bufs=2, space="PSUM"))
            pagg_pool = ctx.enter_context(tc.tile_pool(name="pagg", bufs=3, space="PSUM"))
            pcomb_pool = ctx.enter_context(tc.tile_pool(name="pcomb", bufs=2, space="PSUM"))
            ptr_pool = ctx.enter_context(tc.tile_pool(name="ptr", bufs=1, space="PSUM"))
            tsb_pool = ctx.enter_context(tc.tile_pool(name="tsb", bufs=4))
            msg_pool = ctx.enter_context(tc.tile_pool(name="msg", bufs=4))
            s_pool = ctx.enter_context(tc.tile_pool(name="spool", bufs=3))
            stage_pool = ctx.enter_context(tc.tile_pool(name="stage", bufs=4))
            aggs_pool = ctx.enter_context(tc.tile_pool(name="aggs", bufs=2))
            h_pool = ctx.enter_context(tc.tile_pool(name="hpool", bufs=2))
            o_pool = ctx.enter_context(tc.tile_pool(name="opool", bufs=2))

            # ---- constants
            hTa = cpool.tile([128, kcs[0] * NFREE], f16, name="hTa")
            hT3a = hTa[:].rearrange("p (q n) -> p q n", n=NFREE)
            for r0 in range(0, NFREE, 1664):
                r1 = min(NFREE, r0 + 1664)
                nc.sync.dma_start(hT3a[:, :, r0:r1],
                                  xT_d.rearrange("q p n -> p q n")[:, :, r0:r1])
            hts = [hT3a, hT3a]

            ident = cpool.tile([128, 128], f16, name="ident")
            make_identity(nc, ident[:])
            # iexp[p, h*128 + d] = d   (iota for the S one-hot build)
            iexp = cpool.tile([128, 2 * 128], f16, name="iexp")
            nc.gpsimd.iota(iexp[:], pattern=[[0, 2], [1, 128]], base=0,
                           channel_multiplier=0,
                           allow_small_or_imprecise_dtypes=True)
            ones = cpool.tile([1, 128], f16, name="ones")
            nc.vector.memset(ones[:], 1.0)

            wt = {}
            for l in range(3):
                kd = kcs[l]
                for s in ("l", "r"):
                    t = cpool.tile([128, kd * douts[l]], f16, name=f"w{s}{l}")
                    nc.sync.dma_start(
                        t[:].rearrange("p (q d) -> p q d", d=douts[l]),
                        w_d[(l, s)].rearrange("(q p) d -> p q d", p=128))
                    wt[(l, s)] = t
            bt = []
            for l in range(3):
                t = cpool.tile([1, douts[l]], f16, name=f"bt{l}")
                nc.sync.dma_start(t[:], b_d[l][:, :])
                bt.append(t)

            eidx_t = cpool.tile([128, totc * 16], i16, name="eidxt")
            nc.sync.dma_start(eidx_t[:], eidx_d[:, :])
            dstloc_t = cpool.tile([128, totc * 2], f16, name="dstloct")
            nc.sync.dma_start(dstloc_t[:], dstloc_d[:, :])
            deginv_t = cpool.tile([128, N_CORES * NBLK], f32, name="deginvt")
            nc.sync.dma_start(deginv_t[:], deginv_d[:, :])

            # deginv-scaled psum -> sbuf copies (partials leave pre-scaled, so
            # the post-RS combine needs no deginv pass)
            def _copy_act(dst, srcp, dgcol):
                nc.scalar.activation(dst, srcp,
                                     mybir.ActivationFunctionType.Copy,
                                     scale=dgcol)

            def _copy_dve(dst, srcp, dgcol):
                nc.vector.tensor_scalar_mul(dst, srcp, dgcol)

            copy_engines = [_copy_act, _copy_dve]

            def phase_T(l, b_lo=0, b_hi=NBLK):
                hT3 = hts[l % 2]
                dout = douts[l]
                kc = kcs[l]
                for b in range(b_lo, b_hi):
                    bs = slice(b * 128, (b + 1) * 128)
                    pt = pt_pool.tile([128, dout], f32, tag="pt")
                    for q in range(kc):
                        nc.tensor.matmul(
                            pt[:], lhsT=hT3[:, q, bs],
                            rhs=wt[(l, "l")][:, q * dout:(q + 1) * dout],
                            start=(q == 0), stop=(q == kc - 1))
                    if l < 2:
                        tsb = tsb_pool.tile([128, dout], f8, tag="tsb8")
                        nc.scalar.copy(tsb[:], pt[:])
                        nc.sync.dma_start(t8_d[l][b * 128:(b + 1) * 128, :], tsb[:])
                    else:
                        tsb = tsb_pool.tile([128, 128], f16, tag="tsb2")
                        nc.vector.memset(tsb[:, 64:], 0.0)
                        nc.scalar.copy(tsb[:, :64], pt[:])
                        nc.sync.dma_start(t2_d[b * 128:(b + 1) * 128, :], tsb[:])

            part_writes = {}

            def phase_G(l, p):
                dout = douts[l]
                msgdt = f8 if l < 2 else f16
                elem = dout if l < 2 else 128
                table = t8_d[l] if l < 2 else t2_d
                wgs = [w for w in plan.wgroups if w[0] == p]
                wg_iter = iter(wgs)
                cur_wg = next(wg_iter)
                stage = stage_pool.tile([128, (cur_wg[3] - cur_wg[2]) * dout],
                                        f16, tag="stage")
                for (gp, blo, bhi, c0, nch) in plan.groups:
                    if gp != p:
                        continue
                    msg = msg_pool.tile([128, nch * 2 * elem], msgdt, tag="msg")
                    if "nogather" in dbg:
                        nc.vector.memset(msg[:], 0.25)
                    else:
                        nc.gpsimd.dma_gather(
                            msg[:].rearrange("p (c e) -> p c e", e=elem),
                            table[:, :],
                            eidx_t[:, c0 * 16:(c0 + nch) * 16],
                            num_idxs=nch * 256, num_idxs_reg=nch * 256,
                            elem_size=elem, single_packet=False)
                    # S[p, c, h, d] = (dstloc[p, c, h] == d) over the whole
                    # gather group in one DVE op (f8 for DoubleRow, f16 for l2)
                    S = s_pool.tile([128, nch * 2 * 128], f8 if l < 2 else f16,
                                    tag="S8" if l < 2 else "S16")
                    S4 = S[:].rearrange("p (c h d) -> p c h d", h=2, d=128)
                    dl4 = (dstloc_t[:, c0 * 2:(c0 + nch) * 2]
                           .rearrange("p (c h) -> p c h", h=2)
                           .rearrange("p c (h o) -> p c h o", o=1)
                           .to_broadcast([128, nch, 2, 128]))
                    io4 = (iexp[:].rearrange("(p o) f -> p o f", o=1)
                           .rearrange("p o (h d) -> p o h d", h=2)
                           .to_broadcast([128, nch, 2, 128]))
                    seng = nc.vector
                    seng.tensor_tensor(out=S4, in0=dl4, in1=io4,
                                       op=mybir.AluOpType.is_equal)
                    m3 = msg[:].rearrange("p (c e) -> p c e", e=elem)
                    for bi in range(blo, bhi):
                        bp, bk, bb, bc0, bn = plan.blocks[bi]
                        pagg = pagg_pool.tile([128, dout], f32, tag="pagg")
                        rel = bc0 - c0
                        for j in range(bn):
                            if l < 2:
                                nc.tensor.matmul(
                                    pagg[:], lhsT=S4[:, rel + j],
                                    rhs=m3[:, 2 * (rel + j):2 * (rel + j) + 2, :],
                                    start=(j == 0), stop=(j == bn - 1),
                                    perf_mode=mybir.MatmulPerfMode.DoubleRow)
                            else:
                                for h in range(2):
                                    nc.tensor.matmul(
                                        pagg[:], lhsT=S4[:, rel + j, h],
                                        rhs=m3[:, 2 * (rel + j) + h, :dout],
                                        start=(j == 0 and h == 0),
                                        stop=(j == bn - 1 and h == 1))
                        slot = bi - cur_wg[2]
                        dst = stage[:, slot * dout:(slot + 1) * dout]
                        gcol = bk * NBLK + plan.pfirst[bp] + bb
                        copy_engines[bi % len(copy_engines)](
                            dst, pagg[:], deginv_t[:, gcol:gcol + 1])
                        if bi == cur_wg[3] - 1:
                            nwg = cur_wg[3] - cur_wg[2]
                            _, k, lo, _ = cur_wg
                            b0 = plan.blocks[lo][2]
                            r0 = k * plan.prows[p] + b0 * 128
                            wr = nc.sync.dma_start(
                                part_d[(l, p)][r0:r0 + nwg * 128, :]
                                .rearrange("(g q) d -> q g d", q=128),
                                stage[:].rearrange("p (g d) -> p g d", d=dout))
                            part_writes.setdefault(p, []).append(wr)
                            try:
                                cur_wg = next(wg_iter)
                                stage = stage_pool.tile(
                                    [128, (cur_wg[3] - cur_wg[2]) * dout],
                                    f16, tag="stage")
                            except StopIteration:
                                cur_wg = (p, -1, -1, -1)

            def phase_RS(l, p):
                if "nocoll" in dbg:
                    return
                nc.gpsimd.collective_compute(
                    "ReduceScatter", mybir.AluOpType.add, replica_groups=rg,
                    ins=[part_d[(l, p)].opt()], outs=[agg_d[(l, p)].opt()])

            from concourse.bass import _add_dep_helper

            def phase_C(l, p):
                dout = douts[l]
                kc = kcs[l]
                hT3 = hts[l % 2]
                hT3n = hts[(l + 1) % 2]
                p0 = plan.pfirst[p]
                pb = plan.part_blocks[p]
                if p + 1 < nparts:
                    ws = part_writes.get(p + 1, [])
                    gate = ws[-1] if ws else None
                else:
                    gate = None
                for wlo in range(0, pb, RGW):
                    nwg = min(RGW, pb - wlo)
                    aggsb = aggs_pool.tile([128, nwg * dout], f16, tag="aggsb")
                    rd = nc.sync.dma_start(
                        aggsb[:].rearrange("q (g d) -> q g d", d=dout),
                        agg_d[(l, p)][wlo * 128:(wlo + nwg) * 128, :]
                        .rearrange("(g q) d -> q g d", q=128))
                    if gate is not None:
                        _add_dep_helper(rd.ins, gate.ins, sync=True,
                                        reason="hold combine behind gathers")
                    osb = None
                    if l == 2:
                        osb = o_pool.tile([128, nwg * dout], f32, tag="o")
                    for g in range(nwg):
                        bb = wlo + g
                        b = p0 + bb
                        bs = slice(b * 128, (b + 1) * 128)
                        pm = pcomb_pool.tile([128, dout], f32, tag="pm")
                        for q in range(kc):
                            mm = nc.tensor.matmul(
                                pm[:], lhsT=hT3[:, q, bs],
                                rhs=wt[(l, "r")][:, q * dout:(q + 1) * dout],
                                start=(q == 0), stop=False)
                            if q == 0 and gate is not None:
                                _add_dep_helper(mm.ins, gate.ins, sync=True,
                                                reason="hold combine behind gathers")
                        nc.tensor.matmul(pm[:], lhsT=ones[:1, :], rhs=bt[l][:1, :],
                                         start=False, stop=False)
                        nc.tensor.matmul(pm[:], lhsT=ident[:],
                                         rhs=aggsb[:, g * dout:(g + 1) * dout],
                                         start=False, stop=True)
                        if l < 2:
                            hsb = h_pool.tile([128, dout], f16, tag="h")
                            nc.scalar.activation(hsb[:], pm[:],
                                                 mybir.ActivationFunctionType.Relu)
                            kn = dout // 128
                            ptr = ptr_pool.tile([128, kn * 128], f16, tag="tr")
                            for q in range(kn):
                                nc.tensor.transpose(ptr[:, q * 128:(q + 1) * 128],
                                                    hsb[:, q * 128:(q + 1) * 128],
                                                    ident[:])
                            nc.vector.tensor_copy(
                                hT3n[:, :, bs],
                                ptr[:].rearrange("p (q n) -> p q n", n=128))
                            phase_T(l + 1, b, b + 1)
                        else:
                            nc.scalar.copy(osb[:, g * dout:(g + 1) * dout], pm[:])
                    if l == 2:
                        nc.sync.dma_start(
                            out_d[(p0 + wlo) * 128:(p0 + wlo + nwg) * 128, :]
                            .rearrange("(g q) d -> q g d", q=128),
                            osb[:].rearrange("q (g d) -> q g d", d=dout))

            def phase_CT(l, p):
                phase_C(l, p)

            phase_T(0)
            for l in range(3):
                part_writes.clear()
                for p in range(nparts):
                    phase_G(l, p)
                    phase_RS(l, p)
                for p in range(nparts):
                    phase_CT(l, p)

    nc.compile()
    return nc


# ---------------------------------------------------------------------------

LAST_RESULTS = None
_CACHE = {}


def _make_in_maps(plan, per_core, weights):
    const = {}
    for l, (Wl, Wr, b) in enumerate(weights):
        const[f"wl{l}"] = np.asarray(Wl, np.float32).astype(np.float16)
        const[f"wr{l}"] = np.asarray(Wr, np.float32).astype(np.float16)
        const[f"b{l}"] = np.asarray(b, np.float32).astype(np.float16)[None, :]
    in_maps = []
    for c in range(N_CORES):
        m = dict(const)
        pc = per_core[c]
        m["xT"] = pc["xT"]
        m["eidx"] = pc["eidx"]
        m["dstloc"] = pc["dstloc"]
        m["deginv"] = pc["deginv"]
        in_maps.append(m)
    return in_maps


def _get_program(plan):
    fp = plan.fingerprint
    if fp not in _CACHE:
        _CACHE[fp] = build_program(plan)
    return _CACHE[fp]


def kernel(x, edge_index, relations=None, Wl0=None, Wr0=None, b0=None,
           Wl1=None, Wr1=None, b1=None, Wl2=None, Wr2=None, b2=None, **kw):
    global LAST_RESULTS
    from concourse.bass_utils import run_bass_kernel_spmd

    x = np.asarray(x, np.float32)
    plan, per_core, row = preprocess(x, edge_index)
    nc = _get_program(plan)
    weights = [(Wl0, Wr0, b0), (Wl1, Wr1, b1), (Wl2, Wr2, b2)]
    in_maps = _make_in_maps(plan, per_core, weights)
    import os
    trace = bool(int(os.environ.get("KERNEL_TRACE", "0")))
    res = run_bass_kernel_spmd(nc, in_maps, core_ids=list(range(N_CORES)),
                               trace=trace)
    LAST_RESULTS = res
    out = np.concatenate([res.results[c]["out"] for c in range(N_CORES)], axis=0)
    return out[row].astype(np.float32)



# revision 57
# speedup vs baseline: 1.0051x; 1.0051x over previous
"""3-layer GraphSAGE on 8 trn2 cores — ReduceScatter formulation.

Each core keeps its own node shard's transformed features t = h @ Wl in a
LOCAL DRAM table (fp8 for the two 512-wide layers, fp16x128pad for the
final 64-wide layer), gathers the messages for its OUTGOING edges from
that local table (no cross-core dependency), builds partial
destination-block aggregates for ALL 408 global dst blocks via one-hot
matmuls, writes a [52224, d] fp16 partial table, and a ReduceScatter
(cheap: collective cost scales with its small OUTPUT) sums the 8 partials
and hands each core its own shard's aggregate.  The dst-block space is
split in parts so the RS of part p overlaps the gather/aggregate work of
part p+1; the last part is small so the final (exposed) RS is cheap, and
each part's combine (r-path + inject + relu + transpose) is gated behind
the NEXT part's partial writes so it fills engine idle time inside later
parts' gather windows without stalling the in-order engine queues.

Aggregation runs in fp8 DoubleRow (layers 0/1): 256-edge chunks, one-hot
S built on DVE/GpSimd in f8e4 as [K=128, Ko=2, dst=128] weights against
f8 messages — one PE pass per chunk at 0.5 cycles/row.  Layer 2 keeps
fp16 messages (fp8 there breaks the 2e-2 bar) and consumes the same
256-slot chunk layout as two 128-row fp16 matmuls.  deginv is folded
into the psum->stage copies as a per-partition scale (partitions are dst
nodes), so the post-RS combine needs no separate deginv pass.

Host-side balancer: within each destination shard, nodes are packed into
the 51 dst blocks so that the per-(src-core, block) edge counts stay at
or under 256 (= one DoubleRow chunk).  Without this, the max-over-cores
static chunk count pays a ~1.5x padding tax.
"""

import math
import numpy as np

N_NODES = 50000
N_EDGES = 800000
D_IN = 512
D_HID = 512
D_OUT = 64
N_CORES = 8
NSH = N_NODES // N_CORES          # 6250
NBLK = 51                         # dst blocks per shard (padded: mean 245
                                  # edges per (src-core, block) < 256 cap)
NFREE = NBLK * 128                # 6528
PART_BLOCKS = [13, 13, 13, 9, 3]  # dst-block split per shard
CHUNK = 256                       # edge slots per chunk (fp8 DoubleRow: the
                                  # PE contracts 256 rows per pass)
GATHER_CAP = 8                    # max chunks per dma_gather
WG = 8                            # blocks per batched partial write
RGW = 4                           # blocks per batched agg read (combine)
LOW_LIM_FULL = 32768              # kept for test.py compat (unused)


class Plan:
    pass


def _wrap_idx(v):
    a = np.asarray(v, np.int16).reshape(-1, 16).T   # [16, m/16]
    return np.tile(a, (8, 1))                       # [128, m/16]


def _balance_shard(q, caps):
    """Pack nodes (rows of q: per-src-core in-edge counts) into len(caps)
    blocks so each block's per-core count stays <= 256.  Greedy, heavy
    nodes first.  Returns the new within-shard node order."""
    nb = len(caps)
    nn = q.shape[0]
    loads = np.zeros((nb, q.shape[1]), np.int64)
    cnts = np.zeros(nb, np.int64)
    capv = np.asarray(caps, np.int64)
    order = np.argsort(-q.sum(1), kind="stable")
    assign = np.empty(nn, np.int64)
    for i in order:
        cand = loads + q[i]
        over = np.maximum(cand - 256, 0).sum(1)
        score = over.astype(np.float64) * 1e6 + cand.max(1) \
            + (cnts >= capv) * 1e12
        j = int(np.argmin(score))
        loads[j] += q[i]
        cnts[j] += 1
        assign[i] = j
    assert (cnts <= capv).all() and cnts.sum() == nn
    # local row of each node: bin*128 + rank within bin (blocks may be
    # under-full; pad rows are interspersed)
    o = np.argsort(assign, kind="stable")
    rows = np.empty(nn, np.int64)
    pos = 0
    for b in range(nb):
        c = int(cnts[b])
        rows[o[pos:pos + c]] = b * 128 + np.arange(c)
        pos += c
    return rows


def preprocess(x, edge_index, n_nodes=N_NODES, n_cores=N_CORES, d_in=D_IN,
               low_lim=LOW_LIM_FULL):
    src = np.asarray(edge_index[0], np.int64)
    dst = np.asarray(edge_index[1], np.int64)

    deg = np.bincount(dst, minlength=n_nodes).astype(np.float32)
    deginv_full = (1.0 / np.maximum(deg, 1.0)).astype(np.float32)

    caps = [128] * NBLK

    # ---- balance: assign each node a padded row (shard*NFREE + blk*128+rank)
    csrc_nat = src // NSH
    q = np.zeros((n_nodes, n_cores), np.int64)
    np.add.at(q, (dst, csrc_nat), 1)
    row = np.empty(n_nodes, np.int64)
    for k in range(n_cores):
        rl = _balance_shard(q[k * NSH:(k + 1) * NSH], caps)
        row[k * NSH:(k + 1) * NSH] = k * NFREE + rl

    x = np.asarray(x, np.float32)

    nparts = len(PART_BLOCKS)
    pfirst = np.concatenate([[0], np.cumsum(PART_BLOCKS)]).astype(np.int64)
    prows = [pb * 128 for pb in PART_BLOCKS]

    src_row = row[src]
    dst_row = row[dst]
    kdst = dst_row // NFREE
    r = dst_row - kdst * NFREE
    blocal = r // 128
    within = r - blocal * 128
    part = np.searchsorted(pfirst, blocal, side="right") - 1

    pb_off = np.concatenate([[0], np.cumsum(np.array(PART_BLOCKS) * n_cores)])
    pbs = np.array(PART_BLOCKS)
    gbi = pb_off[part] + kdst * pbs[part] + (blocal - pfirst[part])
    ngb = int(pb_off[-1])

    csrc = src_row // NFREE
    slocal = (src_row - csrc * NFREE).astype(np.int64)

    cnt = np.zeros((n_cores, ngb), np.int64)
    np.add.at(cnt, (csrc, gbi), 1)
    nchunks = np.maximum(-(-cnt.max(axis=0) // CHUNK), 1)
    chunk_base = np.concatenate([[0], np.cumsum(nchunks)]).astype(np.int64)
    totc = int(chunk_base[-1])
    nmax = int(nchunks.max())

    blocks = []   # (part, shard, blk_in_part, chunk0, nchunk)
    i = 0
    for p in range(nparts):
        for k in range(n_cores):
            for b in range(PART_BLOCKS[p]):
                blocks.append((p, k, b, int(chunk_base[i]), int(nchunks[i])))
                i += 1
    assert i == ngb

    groups = []   # (part, blk_lo, blk_hi, chunk0, nchunk)
    gi = 0
    while gi < ngb:
        p = blocks[gi][0]
        lo = gi
        nch = 0
        while gi < ngb and blocks[gi][0] == p and nch + blocks[gi][4] <= GATHER_CAP:
            nch += blocks[gi][4]
            gi += 1
        if gi == lo:
            nch = blocks[gi][4]
            gi += 1
        groups.append((p, lo, gi, blocks[lo][3], nch))

    wgroups = []  # (part, shard, blk_lo, blk_hi)
    gi = 0
    while gi < ngb:
        p, k = blocks[gi][0], blocks[gi][1]
        lo = gi
        while gi < ngb and gi - lo < WG and blocks[gi][0] == p and blocks[gi][1] == k:
            gi += 1
        wgroups.append((p, k, lo, gi))

    plan = Plan()
    plan.nparts = nparts
    plan.part_blocks = list(PART_BLOCKS)
    plan.pfirst = pfirst.tolist()
    plan.prows = prows
    plan.blocks = blocks
    plan.groups = groups
    plan.wgroups = wgroups
    plan.totc = totc
    plan.ngb = ngb
    plan.nmax = nmax
    plan.fingerprint = (tuple(PART_BLOCKS), tuple(nchunks.tolist()),
                        CHUNK, GATHER_CAP, WG)

    # ---- per-core slot tables
    kc = d_in // 128
    per_core = []
    for c in range(n_cores):
        sel = np.nonzero(csrc == c)[0]
        e_g = gbi[sel]
        e_s = slocal[sel]
        e_w = within[sel]
        o = np.argsort(e_g, kind="stable")
        e_g, e_s, e_w = e_g[o], e_s[o], e_w[o]
        grp_start = np.searchsorted(e_g, np.arange(ngb), side="left")
        grp_start_full = np.concatenate([grp_start, [len(e_g)]])
        ranks = np.arange(len(e_g)) - grp_start_full[e_g]
        slots = chunk_base[e_g] * CHUNK + ranks

        idx_slot = np.zeros((totc * CHUNK,), np.int64)
        dst_slot = np.full((totc * CHUNK,), -1.0, np.float32)
        idx_slot[slots] = e_s
        dst_slot[slots] = e_w.astype(np.float32)

        wi = _wrap_idx(idx_slot)                                  # [128, totc*16]
        # dstloc[p, c*2+h] = dst of slot c*CHUNK + h*128 + p   (-1 = empty)
        dl = (dst_slot.reshape(totc, 2, 128).transpose(2, 0, 1)
              .reshape(128, totc * 2).astype(np.float16))

        lrow = row[c * NSH:(c + 1) * NSH] - c * NFREE
        # global deginv table [128, 8*NBLK]: column k*NBLK+b = deginv of the
        # padded node (shard k, block b, partition p); same on every core
        full = np.zeros((n_cores * NFREE,), np.float32)
        full[row] = deginv_full
        dg = full.reshape(n_cores * NBLK, 128).T.astype(np.float32).copy()

        Xp = np.zeros((NFREE, d_in), np.float32)
        Xp[lrow] = x[c * NSH:(c + 1) * NSH]
        xT = np.ascontiguousarray(
            Xp.T.reshape(kc, 128, NFREE).astype(np.float16))

        per_core.append({
            "xT": xT,
            "eidx": np.ascontiguousarray(wi),
            "dstloc": np.ascontiguousarray(dl),
            "deginv": dg,
        })
    return plan, per_core, row


# ---------------------------------------------------------------------------

def build_program(plan, d_in=D_IN, d_hid=D_HID, d_out=D_OUT):
    import os
    dbg = set(os.environ.get("KDBG", "").split(",")) - {""}
    import contextlib
    import concourse.bacc as bacc
    import concourse.tile as tile
    from concourse import mybir
    from concourse.masks import make_identity

    f16 = mybir.dt.float16
    f32 = mybir.dt.float32
    f8 = mybir.dt.float8e4
    i16 = mybir.dt.int16

    nparts = plan.nparts
    totc = plan.totc
    nmax = plan.nmax
    kcs = [d_in // 128, d_hid // 128, d_hid // 128]
    douts = [d_hid, d_hid, d_out]

    nc = bacc.Bacc("TRN2", target_bir_lowering=False, debug=False,
                   num_devices=N_CORES)

    xT_d = nc.dram_tensor("xT", [kcs[0], 128, NFREE], f16, kind="ExternalInput").ap()
    w_d = {}
    for l in range(3):
        kd = [d_in, d_hid, d_hid][l]
        for s in ("l", "r"):
            w_d[(l, s)] = nc.dram_tensor(f"w{s}{l}", [kd, douts[l]], f16,
                                         kind="ExternalInput").ap()
    b_d = [nc.dram_tensor(f"b{l}", [1, douts[l]], f16, kind="ExternalInput").ap()
           for l in range(3)]
    eidx_d = nc.dram_tensor("eidx", [128, totc * 16], i16, kind="ExternalInput").ap()
    dstloc_d = nc.dram_tensor("dstloc", [128, totc * 2], f16, kind="ExternalInput").ap()
    deginv_d = nc.dram_tensor("deginv", [128, N_CORES * NBLK], f32,
                              kind="ExternalInput").ap()
    out_d = nc.dram_tensor("out", [NFREE, d_out], f32, kind="ExternalOutput").ap()

    t8_d = [nc.dram_tensor(f"t8_{l}", [NFREE, d_hid], f8, kind="Internal").ap()
            for l in range(2)]
    t2_d = nc.dram_tensor("t2", [NFREE, 128], f16, kind="Internal").ap()

    part_d = {}
    agg_d = {}
    for l in range(3):
        dd = douts[l]
        for p in range(nparts):
            rows = plan.prows[p]
            part_d[(l, p)] = nc.dram_tensor(
                f"part{l}_{p}", [N_CORES * rows, dd], f16, kind="Internal").ap()
            agg_d[(l, p)] = nc.dram_tensor(
                f"agg{l}_{p}", [rows, dd], f16, kind="Internal").ap()

    rg = [list(range(N_CORES))]

    with tile.TileContext(nc) as tc:
        with contextlib.ExitStack() as ctx:
            cpool = ctx.enter_context(tc.tile_pool(name="const", bufs=1))
            pt_pool = ctx.enter_context(tc.tile_pool(name="pt", bufs=2, space="PSUM"))
            pagg_pool = ctx.enter_context(tc.tile_pool(name="pagg", bufs=3, space="PSUM"))
            pcomb_pool = ctx.enter_context(tc.tile_pool(name="pcomb", bufs=2, space="PSUM"))
            ptr_pool = ctx.enter_context(tc.tile_pool(name="ptr", bufs=1, space="PSUM"))
            tsb_pool = ctx.enter_context(tc.tile_pool(name="tsb", bufs=3))
            msg_pool = ctx.enter_context(tc.tile_pool(name="msg", bufs=4))
            s_pool = ctx.enter_context(tc.tile_pool(name="spool", bufs=3))
            stage_pool = ctx.enter_context(tc.tile_pool(name="stage", bufs=4))
            aggs_pool = ctx.enter_context(tc.tile_pool(name="aggs", bufs=2))
            h_pool = ctx.enter_context(tc.tile_pool(name="hpool", bufs=2))
            o_pool = ctx.enter_context(tc.tile_pool(name="opool", bufs=2))

            # ---- constants
            hTa = cpool.tile([128, kcs[0] * NFREE], f16, name="hTa")
            hT3a = hTa[:].rearrange("p (q n) -> p q n", n=NFREE)
            for r0 in range(0, NFREE, 1664):
                r1 = min(NFREE, r0 + 1664)
                nc.sync.dma_start(hT3a[:, :, r0:r1],
                                  xT_d.rearrange("q p n -> p q n")[:, :, r0:r1])
            hts = [hT3a, hT3a]

            ident = cpool.tile([128, 128], f16, name="ident")
            make_identity(nc, ident[:])
            # iexp[p, h*128 + d] = d   (iota for the S one-hot build)
            iexp = cpool.tile([128, 2 * 128], f16, name="iexp")
            nc.gpsimd.iota(iexp[:], pattern=[[0, 2], [1, 128]], base=0,
                           channel_multiplier=0,
                           allow_small_or_imprecise_dtypes=True)
            ones = cpool.tile([1, 128], f16, name="ones")
            nc.vector.memset(ones[:], 1.0)

            wt = {}
            for l in range(3):
                kd = kcs[l]
                for s in ("l", "r"):
                    t = cpool.tile([128, kd * douts[l]], f16, name=f"w{s}{l}")
                    nc.sync.dma_start(
                        t[:].rearrange("p (q d) -> p q d", d=douts[l]),
                        w_d[(l, s)].rearrange("(q p) d -> p q d", p=128))
                    wt[(l, s)] = t
            bt = []
            for l in range(3):
                t = cpool.tile([1, douts[l]], f16, name=f"bt{l}")
                nc.sync.dma_start(t[:], b_d[l][:, :])
                bt.append(t)

            eidx_t = cpool.tile([128, totc * 16], i16, name="eidxt")
            nc.sync.dma_start(eidx_t[:], eidx_d[:, :])
            dstloc_t = cpool.tile([128, totc * 2], f16, name="dstloct")
            nc.sync.dma_start(dstloc_t[:], dstloc_d[:, :])
            deginv_t = cpool.tile([128, N_CORES * NBLK], f32, name="deginvt")
            nc.sync.dma_start(deginv_t[:], deginv_d[:, :])

            # deginv-scaled psum -> sbuf copies (partials leave pre-scaled, so
            # the post-RS combine needs no deginv pass)
            def _copy_act(dst, srcp, dgcol):
                nc.scalar.activation(dst, srcp,
                                     mybir.ActivationFunctionType.Copy,
                                     scale=dgcol)

            def _copy_dve(dst, srcp, dgcol):
                nc.vector.tensor_scalar_mul(dst, srcp, dgcol)

            copy_engines = [_copy_act, _copy_dve]

            def phase_T(l, b_lo=0, b_hi=NBLK):
                hT3 = hts[l % 2]
                dout = douts[l]
                kc = kcs[l]
                for b in range(b_lo, b_hi):
                    bs = slice(b * 128, (b + 1) * 128)
                    pt = pt_pool.tile([128, dout], f32, tag="pt")
                    for q in range(kc):
                        nc.tensor.matmul(
                            pt[:], lhsT=hT3[:, q, bs],
                            rhs=wt[(l, "l")][:, q * dout:(q + 1) * dout],
                            start=(q == 0), stop=(q == kc - 1))
                    if l < 2:
                        tsb = tsb_pool.tile([128, dout], f8, tag="tsb8")
                        nc.scalar.copy(tsb[:], pt[:])
                        nc.sync.dma_start(t8_d[l][b * 128:(b + 1) * 128, :], tsb[:])
                    else:
                        tsb = tsb_pool.tile([128, 128], f16, tag="tsb2")
                        nc.vector.memset(tsb[:, 64:], 0.0)
                        nc.scalar.copy(tsb[:, :64], pt[:])
                        nc.sync.dma_start(t2_d[b * 128:(b + 1) * 128, :], tsb[:])

            part_writes = {}

            def phase_G(l, p):
                dout = douts[l]
                msgdt = f8 if l < 2 else f16
                elem = dout if l < 2 else 128
                table = t8_d[l] if l < 2 else t2_d
                wgs = [w for w in plan.wgroups if w[0] == p]
                wg_iter = iter(wgs)
                cur_wg = next(wg_iter)
                stage = stage_pool.tile([128, (cur_wg[3] - cur_wg[2]) * dout],
                                        f16, tag="stage")
                for (gp, blo, bhi, c0, nch) in plan.groups:
                    if gp != p:
                        continue
                    msg = msg_pool.tile([128, nch * 2 * elem], msgdt, tag="msg")
                    if "nogather" in dbg:
                        nc.vector.memset(msg[:], 0.25)
                    else:
                        nc.gpsimd.dma_gather(
                            msg[:].rearrange("p (c e) -> p c e", e=elem),
                            table[:, :],
                            eidx_t[:, c0 * 16:(c0 + nch) * 16],
                            num_idxs=nch * 256, num_idxs_reg=nch * 256,
                            elem_size=elem, single_packet=False)
                    # S[p, c, h, d] = (dstloc[p, c, h] == d) over the whole
                    # gather group in one DVE op (f8 for DoubleRow, f16 for l2)
                    S = s_pool.tile([128, nch * 2 * 128], f8 if l < 2 else f16,
                                    tag="S8" if l < 2 else "S16")
                    S4 = S[:].rearrange("p (c h d) -> p c h d", h=2, d=128)
                    dl4 = (dstloc_t[:, c0 * 2:(c0 + nch) * 2]
                           .rearrange("p (c h) -> p c h", h=2)
                           .rearrange("p c (h o) -> p c h o", o=1)
                           .to_broadcast([128, nch, 2, 128]))
                    io4 = (iexp[:].rearrange("(p o) f -> p o f", o=1)
                           .rearrange("p o (h d) -> p o h d", h=2)
                           .to_broadcast([128, nch, 2, 128]))
                    seng = nc.vector
                    seng.tensor_tensor(out=S4, in0=dl4, in1=io4,
                                       op=mybir.AluOpType.is_equal)
                    m3 = msg[:].rearrange("p (c e) -> p c e", e=elem)
                    for bi in range(blo, bhi):
                        bp, bk, bb, bc0, bn = plan.blocks[bi]
                        pagg = pagg_pool.tile([128, dout], f32, tag="pagg")
                        rel = bc0 - c0
                        for j in range(bn):
                            if l < 2:
                                nc.tensor.matmul(
                                    pagg[:], lhsT=S4[:, rel + j],
                                    rhs=m3[:, 2 * (rel + j):2 * (rel + j) + 2, :],
                                    start=(j == 0), stop=(j == bn - 1),
                                    perf_mode=mybir.MatmulPerfMode.DoubleRow)
                            else:
                                for h in range(2):
                                    nc.tensor.matmul(
                                        pagg[:], lhsT=S4[:, rel + j, h],
                                        rhs=m3[:, 2 * (rel + j) + h, :dout],
                                        start=(j == 0 and h == 0),
                                        stop=(j == bn - 1 and h == 1))
                        slot = bi - cur_wg[2]
                        dst = stage[:, slot * dout:(slot + 1) * dout]
                        gcol = bk * NBLK + plan.pfirst[bp] + bb
                        copy_engines[bi % len(copy_engines)](
                            dst, pagg[:], deginv_t[:, gcol:gcol + 1])
                        if bi == cur_wg[3] - 1:
                            nwg = cur_wg[3] - cur_wg[2]
                            _, k, lo, _ = cur_wg
                            b0 = plan.blocks[lo][2]
                            r0 = k * plan.prows[p] + b0 * 128
                            wr = nc.sync.dma_start(
                                part_d[(l, p)][r0:r0 + nwg * 128, :]
                                .rearrange("(g q) d -> q g d", q=128),
                                stage[:].rearrange("p (g d) -> p g d", d=dout))
                            part_writes.setdefault(p, []).append(wr)
                            try:
                                cur_wg = next(wg_iter)
                                stage = stage_pool.tile(
                                    [128, (cur_wg[3] - cur_wg[2]) * dout],
                                    f16, tag="stage")
                            except StopIteration:
                                cur_wg = (p, -1, -1, -1)

            def phase_RS(l, p):
                if "nocoll" in dbg:
                    return
                nc.gpsimd.collective_compute(
                    "ReduceScatter", mybir.AluOpType.add, replica_groups=rg,
                    ins=[part_d[(l, p)].opt()], outs=[agg_d[(l, p)].opt()])

            from concourse.bass import _add_dep_helper

            def phase_C(l, p):
                dout = douts[l]
                kc = kcs[l]
                hT3 = hts[l % 2]
                hT3n = hts[(l + 1) % 2]
                p0 = plan.pfirst[p]
                pb = plan.part_blocks[p]
                if p + 1 < nparts:
                    ws = part_writes.get(p + 1, [])
                    gate = ws[-1] if ws else None
                else:
                    gate = None
                for wlo in range(0, pb, RGW):
                    nwg = min(RGW, pb - wlo)
                    aggsb = aggs_pool.tile([128, nwg * dout], f16, tag="aggsb")
                    rd = nc.sync.dma_start(
                        aggsb[:].rearrange("q (g d) -> q g d", d=dout),
                        agg_d[(l, p)][wlo * 128:(wlo + nwg) * 128, :]
                        .rearrange("(g q) d -> q g d", q=128))
                    if gate is not None:
                        _add_dep_helper(rd.ins, gate.ins, sync=True,
                                        reason="hold combine behind gathers")
                    osb = None
                    if l == 2:
                        osb = o_pool.tile([128, nwg * dout], f32, tag="o")
                    for g in range(nwg):
                        bb = wlo + g
                        b = p0 + bb
                        bs = slice(b * 128, (b + 1) * 128)
                        pm = pcomb_pool.tile([128, dout], f32, tag="pm")
                        for q in range(kc):
                            mm = nc.tensor.matmul(
                                pm[:], lhsT=hT3[:, q, bs],
                                rhs=wt[(l, "r")][:, q * dout:(q + 1) * dout],
                                start=(q == 0), stop=False)
                            if q == 0 and gate is not None:
                                _add_dep_helper(mm.ins, gate.ins, sync=True,
                                                reason="hold combine behind gathers")
                        nc.tensor.matmul(pm[:], lhsT=ones[:1, :], rhs=bt[l][:1, :],
                                         start=False, stop=False)
                        nc.tensor.matmul(pm[:], lhsT=ident[:],
                                         rhs=aggsb[:, g * dout:(g + 1) * dout],
                                         start=False, stop=True)
                        if l < 2:
                            hsb = h_pool.tile([128, dout], f16, tag="h")
                            nc.scalar.activation(hsb[:], pm[:],
                                                 mybir.ActivationFunctionType.Relu)
                            kn = dout // 128
                            ptr = ptr_pool.tile([128, kn * 128], f16, tag="tr")
                            for q in range(kn):
                                nc.tensor.transpose(ptr[:, q * 128:(q + 1) * 128],
                                                    hsb[:, q * 128:(q + 1) * 128],
                                                    ident[:])
                            nc.vector.tensor_copy(
                                hT3n[:, :, bs],
                                ptr[:].rearrange("p (q n) -> p q n", n=128))
                            phase_T(l + 1, b, b + 1)
                        else:
                            nc.scalar.copy(osb[:, g * dout:(g + 1) * dout], pm[:])
                    if l == 2:
                        nc.sync.dma_start(
                            out_d[(p0 + wlo) * 128:(p0 + wlo + nwg) * 128, :]
                            .rearrange("(g q) d -> q g d", q=128),
                            osb[:].rearrange("q (g d) -> q g d", d=dout))

            def phase_CT(l, p):
                phase_C(l, p)

            phase_T(0)
            for l in range(3):
                part_writes.clear()
                for p in range(nparts):
                    phase_G(l, p)
                    phase_RS(l, p)
                for p in range(nparts):
                    phase_CT(l, p)

    nc.compile()
    return nc


# ---------------------------------------------------------------------------

LAST_RESULTS = None
_CACHE = {}


def _make_in_maps(plan, per_core, weights):
    const = {}
    for l, (Wl, Wr, b) in enumerate(weights):
        const[f"wl{l}"] = np.asarray(Wl, np.float32).astype(np.float16)
        const[f"wr{l}"] = np.asarray(Wr, np.float32).astype(np.float16)
        const[f"b{l}"] = np.asarray(b, np.float32).astype(np.float16)[None, :]
    in_maps = []
    for c in range(N_CORES):
        m = dict(const)
        pc = per_core[c]
        m["xT"] = pc["xT"]
        m["eidx"] = pc["eidx"]
        m["dstloc"] = pc["dstloc"]
        m["deginv"] = pc["deginv"]
        in_maps.append(m)
    return in_maps


def _get_program(plan):
    fp = plan.fingerprint
    if fp not in _CACHE:
        _CACHE[fp] = build_program(plan)
    return _CACHE[fp]


def kernel(x, edge_index, relations=None, Wl0=None, Wr0=None, b0=None,
           Wl1=None, Wr1=None, b1=None, Wl2=None, Wr2=None, b2=None, **kw):
    global LAST_RESULTS
    from concourse.bass_utils import run_bass_kernel_spmd

    x = np.asarray(x, np.float32)
    plan, per_core, row = preprocess(x, edge_index)
    nc = _get_program(plan)
    weights = [(Wl0, Wr0, b0), (Wl1, Wr1, b1), (Wl2, Wr2, b2)]
    in_maps = _make_in_maps(plan, per_core, weights)
    import os
    trace = bool(int(os.environ.get("KERNEL_TRACE", "0")))
    res = run_bass_kernel_spmd(nc, in_maps, core_ids=list(range(N_CORES)),
                               trace=trace)
    LAST_RESULTS = res
    out = np.concatenate([res.results[c]["out"] for c in range(N_CORES)], axis=0)
    return out[row].astype(np.float32)

